# revision 1
# baseline (speedup 1.0000x reference)
"""Trainium2 Bass kernel for CoordLSVotingWeighted (segment_reduce).

Strategy: data-parallel over batch B=8 across 8 NeuronCores (1 image/core).
Per image, on device:
  - hard one-hot of argmax over 9 seg channels (matches softmax(seg*1e6))
  - unit-direction projection matrices R = w*(I - n n^T), w = softplus(w)
  - segment-reduce per class via TensorE matmul:
      psum[24,27] = sum_pix lhsT[pix, {hot, hot*ch, hot*cw}]^T
                    @ rhs[pix, {R00, m, R11}]   (m = -R01 = w*nx*ny/s)
Host: assemble 2x2 systems in float64, pinv-solve, scale by HEIGHT.

Self-contained: only needs numpy / ml_dtypes / concourse (installed env).
"""

import os

import numpy as np

B = 8
H = 128
W = 128
NCLS = 9  # seg channels, class 0 = background
NPTS = 9
OC = 8
HEIGHT = 128.0
N_CORES = 8

_cache: dict = {}


def _build_nc():
    import concourse.bacc as bacc
    import concourse.tile as tile
    import concourse.mybir as mybir
    from concourse.alu_op_type import AluOpType as Alu

    Act = mybir.ActivationFunctionType
    Axis = mybir.AxisListType
    f32 = mybir.dt.float32
    b16 = mybir.dt.bfloat16

    nc = bacc.Bacc(
        "TRN2", target_bir_lowering=False, debug=False, num_devices=N_CORES
    )
    seg_d = nc.dram_tensor("seg", [H, W * NCLS], f32, kind="ExternalInput")
    dct_d = nc.dram_tensor("direct", [H, W * NPTS * 2], f32, kind="ExternalInput")
    w_d = nc.dram_tensor("w", [H, W * NPTS], f32, kind="ExternalInput")
    cw_d = nc.dram_tensor("cw8", [H, OC * W], b16, kind="ExternalInput")
    ch_d = nc.dram_tensor("chv", [H, 1], f32, kind="ExternalInput")
    out_d = nc.dram_tensor("acc", [3 * OC, 3 * NPTS], f32, kind="ExternalOutput")

    NF = W * NPTS  # 1152

    with tile.TileContext(nc) as tc:
        with (
            tc.tile_pool(name="main", bufs=1) as pool,
            tc.tile_pool(name="ps", bufs=1, space="PSUM") as psp,
        ):
            # ---- input tiles
            sgt = pool.tile([H, W * NCLS], f32, tag="sgt")
            dct = pool.tile([H, W * NPTS * 2], f32, tag="dct")
            wdt = pool.tile([H, W * NPTS], f32, tag="wdt")
            cwt = pool.tile([H, OC * W], b16, tag="cwt")
            cht = pool.tile([H, 1], f32, tag="cht")
            # two DMA queues in parallel: {w, seg, ch} on sync, {direct, cw} on vector
            nc.sync.dma_start(out=wdt[:, :], in_=w_d[:, :])
            nc.sync.dma_start(out=dct[:, :], in_=dct_d[:, :])
            nc.sync.dma_start(out=sgt[:, :], in_=seg_d[:, :])
            nc.sync.dma_start(out=cwt[:, :], in_=cw_d[:, :])
            nc.sync.dma_start(out=cht[:, :], in_=ch_d[:, :])

            # ---- work tiles (bf16 unless noted)
            sq = pool.tile([H, 2 * NF], b16, tag="sq")     # [x^2|y^2]
            s16 = pool.tile([H, NF], b16, tag="s16")
            ls32 = pool.tile([H, NF], f32, tag="ls32")
            rr16 = pool.tile([H, NF], b16, tag="rr16")
            ew16 = pool.tile([H, NF], b16, tag="ew16")
            sp16 = pool.tile([H, NF], b16, tag="sp16")
            k16 = pool.tile([H, NF], b16, tag="k16")
            u16 = pool.tile([H, NF], b16, tag="u16")
            mx = pool.tile([H, W], f32, tag="mx")
            b9 = pool.tile([H, 1], f32, tag="b9")
            nc.vector.memset(b9[:, :], 1e-9)
            L = pool.tile([H, 3 * OC * W], b16, tag="L")   # hot|hot*ch|hot*cw
            R = pool.tile([H, 3 * NF], b16, tag="R")       # R00|m|R11
            outs = pool.tile([3 * OC, 3 * NPTS], f32, tag="outs")

            # ---- one-hot lhs first: depends only on seg DMA
            sgt_wc = sgt[:, :].rearrange("q (w c) -> q w c", c=NCLS)
            nc.vector.tensor_reduce(
                out=mx[:, :], in_=sgt_wc, axis=Axis.X, op=Alu.max
            )
            sgt_cw = sgt[:, :].rearrange("q (w c) -> q c w", c=NCLS)[:, 1:NCLS, :]
            mx_b = mx[:, :].unsqueeze(1).broadcast_to((H, OC, W))
            hot_r = L[:, 0 : OC * W].rearrange("q (c w) -> q c w", c=OC)
            nc.vector.tensor_tensor(
                out=hot_r, in0=sgt_cw, in1=mx_b, op=Alu.is_equal
            )
            nc.vector.tensor_scalar_mul(
                L[:, OC * W : 2 * OC * W], L[:, 0 : OC * W], cht[:, :]
            )
            nc.vector.tensor_tensor(
                out=L[:, 2 * OC * W : 3 * OC * W], in0=L[:, 0 : OC * W],
                in1=cwt[:, :], op=Alu.mult,
            )

            # ---- strided views of direct: nx = even cols, ny = odd cols
            nxs = dct[:, 0::2].rearrange("q (w g) -> q g w", g=NPTS)
            nys = dct[:, 1::2].rearrange("q (w g) -> q g w", g=NPTS)

            # ---- softplus(w) = Ln(Exp(w) + 1), single ACT table set (ln/exp)
            w_r = wdt[:, :].rearrange("q (w g) -> q g w", g=NPTS)
            ew_r = ew16[:, :].rearrange("q (g w) -> q g w", g=NPTS)
            nc.scalar.activation(out=ew_r, in_=w_r, func=Act.Exp)
            nc.scalar.activation(out=sp16[:, :], in_=ew16[:, :], func=Act.Ln, bias=1.0)

            # ---- squares via ACT (Square is in the resident table set)
            sqx_r = sq[:, 0:NF].rearrange("q (g w) -> q g w", g=NPTS)
            sqy_r = sq[:, NF : 2 * NF].rearrange("q (g w) -> q g w", g=NPTS)
            nc.vector.tensor_tensor(out=sqx_r, in0=nxs, in1=nxs, op=Alu.mult)
            nc.vector.tensor_tensor(out=sqy_r, in0=nys, in1=nys, op=Alu.mult)
            nc.vector.tensor_tensor(
                out=s16[:, :], in0=sq[:, 0:NF], in1=sq[:, NF : 2 * NF], op=Alu.add
            )
            nc.scalar.activation(
                out=ls32[:, :], in_=s16[:, :], func=Act.Ln, bias=b9[:, :]
            )
            nc.scalar.activation(out=rr16[:, :], in_=ls32[:, :], func=Act.Exp, scale=-1.0)

            # ---- k = softplus(w)/s ; rhs features R00=k*ny^2, m=k*nx*ny, R11=k*nx^2
            nc.vector.tensor_tensor(
                out=k16[:, :], in0=sp16[:, :], in1=rr16[:, :], op=Alu.mult
            )
            nc.vector.tensor_tensor(
                out=R[:, 0:NF], in0=k16[:, :], in1=sq[:, NF : 2 * NF], op=Alu.mult
            )
            k16_r = k16[:, :].rearrange("q (g w) -> q g w", g=NPTS)
            u16_r = u16[:, :].rearrange("q (g w) -> q g w", g=NPTS)
            nc.vector.tensor_tensor(out=u16_r, in0=k16_r, in1=nxs, op=Alu.mult)
            nc.vector.tensor_tensor(
                out=R[:, NF : 2 * NF].rearrange("q (g w) -> q g w", g=NPTS),
                in0=u16_r, in1=nys, op=Alu.mult,
            )
            nc.vector.tensor_tensor(
                out=R[:, 2 * NF : 3 * NF], in0=k16[:, :], in1=sq[:, 0:NF], op=Alu.mult
            )

            # ---- segment reduce: 128 accumulating matmuls over w-chunks
            acc = psp.tile([3 * OC, 3 * NPTS], f32, tag="acc")
            for wi in range(W):
                nc.tensor.matmul(
                    acc[:, :],
                    L[:, wi :: W],
                    R[:, wi :: W],
                    start=(wi == 0),
                    stop=(wi == W - 1),
                )

            nc.vector.tensor_copy(out=outs[:, :], in_=acc[:, :])
            nc.sync.dma_start(out=out_d[:, :], in_=outs[:, :])

    nc.compile()
    return nc


def _host_constants():
    import ml_dtypes

    bf16 = ml_dtypes.bfloat16
    coord = ((np.arange(128, dtype=np.float32) + 0.5) / HEIGHT).astype(bf16)
    cw8 = np.ascontiguousarray(
        np.broadcast_to(coord[None, None, :], (H, OC, W))
    ).reshape(H, OC * W)
    chv = ((np.arange(128, dtype=np.float32) + 0.5) / HEIGHT).reshape(H, 1)
    return cw8, chv


def _solve_host(acc_f32: np.ndarray) -> np.ndarray:
    """acc [24,27] fp32 -> p [OC, NPTS, 2] fp32 (float64 pinv like reference)."""
    a = acc_f32.astype(np.float64)
    A = a[0:OC, 0:9]
    Bm = a[0:OC, 9:18]
    D = a[0:OC, 18:27]
    S1 = a[OC : 2 * OC, 0:9]
    S3 = a[OC : 2 * OC, 9:18]
    S2 = a[2 * OC : 3 * OC, 9:18]
    S4 = a[2 * OC : 3 * OC, 18:27]
    Rm = np.empty((OC, NPTS, 2, 2), dtype=np.float64)
    Rm[..., 0, 0] = A
    Rm[..., 0, 1] = -Bm
    Rm[..., 1, 0] = -Bm
    Rm[..., 1, 1] = D
    q = np.stack([S1 - S2, S4 - S3], axis=-1)
    Rp = np.linalg.pinv(Rm.reshape(-1, 2, 2)).reshape(Rm.shape)
    p = np.einsum("cpij,cpj->cpi", Rp, q) * HEIGHT
    return p.astype(np.float32)


def kernel(seg, direct, w):
    if "nc" not in _cache:
        _cache["nc"] = _build_nc()
    nc = _cache["nc"]

    seg = np.ascontiguousarray(np.asarray(seg, dtype=np.float32))
    direct = np.ascontiguousarray(np.asarray(direct, dtype=np.float32))
    w = np.ascontiguousarray(np.asarray(w, dtype=np.float32))
    cw8, chv = _host_constants()

    in_maps = []
    for i in range(B):
        in_maps.append(
            {
                "seg": seg[i].reshape(H, W * NCLS),
                "direct": direct[i].reshape(H, W * NPTS * 2),
                "w": w[i].reshape(H, W * NPTS),
                "cw8": cw8,
                "chv": chv,
            }
        )

    from concourse.bass_utils import run_bass_kernel_spmd

    trace = bool(int(os.environ.get("KERNEL_TRACE", "0")))
    res = run_bass_kernel_spmd(
        nc, in_maps, core_ids=list(range(N_CORES)), trace=trace
    )
    kernel._last_exec_ns = res.exec_time_ns
    kernel._last_results = res

    out = np.stack(
        [_solve_host(np.asarray(res.results[i]["acc"])) for i in range(B)], axis=0
    )
    return out



# revision 3
# speedup vs baseline: 1.4790x; 1.4790x over previous
"""Trainium2 Bass kernel for CoordLSVotingWeighted (segment_reduce).

Strategy: data-parallel over batch B=8 across 8 NeuronCores (1 image/core).

Per image, on device (w-chunked pipeline, NCH chunks):
  - hard one-hot of argmax over 9 seg channels (matches softmax(seg*1e6))
  - unit-direction projection features via a custom fused DVE op:
      rinv = approx 1/(nx^2+ny^2)   (bitwise-NOT seed + 1 Newton step)
      t = softplus(w)*rinv ; u = t*nx ; m = u*ny ; R11 = u*nx
    softplus on the scalar engine (Exp then Ln, one ACT table set);
    R00 is never materialized: R00 = sp - R11, recovered on host from
    the sp-feature accumulators.
  - segment reduce on TensorE, G=4 w-columns packed per matmul:
      lhsT = interleaved L [w, {hot, hot*ch, hot*cw}, class]  (96 cols)
      rhs  = planar R [w-window, {sp, m, R11}, point]         (108 cols)
      PSUM [96, 108] accumulates; diagonal 24x27 blocks summed on host.
Host: assemble 2x2 systems in float64, pinv-solve, scale by HEIGHT.

Self-contained: only needs numpy / ml_dtypes / concourse (installed env).
"""

import os

import numpy as np

B = 8
H = 128
W = 128
NCLS = 9  # seg channels, class 0 = background
NPTS = 9
OC = 8
HEIGHT = 128.0
N_CORES = 8

NCH = 2          # w-chunks
WC = W // NCH    # 64 w columns per chunk
G = 4            # w columns packed per matmul
NFC = NPTS * WC  # 576 point-cols per chunk

# 1-Newton reciprocal-approx constants (minimax over s in [1e-8, 1e8])
RC0 = -0.2355
RC1 = 2.0015

_cache: dict = {}


def _register_rinv():
    """Runtime-register the custom DVE op RINV_XY = recip1(x^2 + y^2)."""
    import concourse.dve_ops as dops
    from concourse.dve_spec import (
        Spec, Src0, Src1, C0, C1, AluOp, Bin, lower, _has_src1,
    )
    from concourse.dve_uop import DveOpSpec

    for o in dops.OPS:
        if o.name == "RINV_XY":
            return o

    s = Src0 * Src0 + Src1 * Src1
    nb = Bin(AluOp.BITWISE_NOT, s, s)
    y0 = nb * C0
    y1 = y0 * (C1 - s * y0)

    def _ref(in0, in1, s0, s1, imm2):
        ss = (in0.astype(np.float32) ** 2 + in1.astype(np.float32) ** 2).astype(
            np.float32
        )
        nbv = (~ss.view(np.int32)).view(np.float32)
        y0v = (nbv * np.float32(s0)).astype(np.float32)
        return (y0v * (np.float32(s1) - ss * y0v)).astype(np.float32)

    spec = Spec(body=y1, reference=_ref)
    opcode = dops._CUSTOM_DVE_ROW_BASE + len(dops.OPS)
    shas = {}
    for ver in ("v3", "v4"):
        try:
            shas[ver] = DveOpSpec(
                name="RINV_XY",
                opcode=opcode,
                uops=lower(spec, ver=ver),
                rd1_en=_has_src1(spec),
            ).sha(ver)
        except Exception:
            pass
    op = dops.DveOp("RINV_XY", spec, subdim=False, uops_sha=shas)
    dops.OPS.append(op)
    dops.CUSTOM_DVE_SPECS[op.name] = op.spec
    dops._SUB_OPCODE_FOR_NAME[op.name] = opcode
    return op


def _patch_act_tables():
    """Exp and Ln resolve only to natural_log_exp_and_others -> 1 table load."""
    import concourse.bacc as bacc
    import concourse.mybir as mybir

    A = mybir.ActivationFunctionType
    orig = bacc.get_activation_tables
    if getattr(orig, "_softplus_patched", False):
        return

    def patched(arch):
        out = {}
        for name, funcs in orig(arch).items():
            f = set(funcs)
            if name != "natural_log_exp_and_others":
                f.discard(A.Exp)
                f.discard(A.Ln)
            out[name] = f
        return out

    patched._softplus_patched = True
    bacc.get_activation_tables = patched


def _build_nc():
    import concourse.bacc as bacc
    import concourse.tile as tile
    import concourse.mybir as mybir
    from concourse.alu_op_type import AluOpType as Alu

    Act = mybir.ActivationFunctionType
    Axis = mybir.AxisListType
    f32 = mybir.dt.float32
    b16 = mybir.dt.bfloat16

    RINV = _register_rinv()
    _patch_act_tables()

    nc = bacc.Bacc(
        "TRN2", target_bir_lowering=False, debug=False, num_devices=N_CORES
    )
    # chunk-major layouts (each chunk's block contiguous in columns)
    seg_d = nc.dram_tensor("seg", [H, W * NCLS], f32, kind="ExternalInput")
    nyx_d = nc.dram_tensor("nyx", [H, NCH * 2 * NFC], b16, kind="ExternalInput")
    w_d = nc.dram_tensor("w", [H, NCH * NFC], b16, kind="ExternalInput")
    cwb_d = nc.dram_tensor("cwb", [H, W * OC], b16, kind="ExternalInput")
    ch_d = nc.dram_tensor("chv", [H, 1], f32, kind="ExternalInput")
    out_d = nc.dram_tensor("acc", [G * 24, G * 27], f32, kind="ExternalOutput")

    with tile.TileContext(nc) as tc:
        with (
            tc.tile_pool(name="main", bufs=1) as pool,
            tc.tile_pool(name="ps", bufs=1, space="PSUM") as psp,
        ):
            cht = pool.tile([H, 1], f32, tag="cht")
            nc.sync.dma_start(out=cht[:, :], in_=ch_d[:, :])

            sgt = [pool.tile([H, WC * NCLS], f32, tag=f"sgt{c}", name=f"sgt{c}") for c in range(NCH)]
            nyxt = [pool.tile([H, 2 * NFC], b16, tag=f"nyxt{c}", name=f"nyxt{c}") for c in range(NCH)]
            wtt = [pool.tile([H, NFC], b16, tag=f"wtt{c}", name=f"wtt{c}") for c in range(NCH)]
            cwbt = [pool.tile([H, WC * OC], b16, tag=f"cwbt{c}", name=f"cwbt{c}") for c in range(NCH)]
            # issue all input DMAs up front (chunk-major slices are contiguous)
            for c in range(NCH):
                nc.sync.dma_start(
                    out=sgt[c][:, :],
                    in_=seg_d[:, c * WC * NCLS : (c + 1) * WC * NCLS],
                )
                nc.sync.dma_start(
                    out=wtt[c][:, :], in_=w_d[:, c * NFC : (c + 1) * NFC]
                )
                nc.sync.dma_start(
                    out=nyxt[c][:, :],
                    in_=nyx_d[:, c * 2 * NFC : (c + 1) * 2 * NFC],
                )
                nc.sync.dma_start(
                    out=cwbt[c][:, :], in_=cwb_d[:, c * WC * OC : (c + 1) * WC * OC]
                )

            acc = psp.tile([G * 24, G * 27], f32, tag="acc")

            for c in range(NCH):
                mxt = pool.tile([H, WC], f32, tag=f"mx{c}")
                ewt = pool.tile([H, NFC], b16, tag=f"ew{c}")
                rit = pool.tile([H, NFC], b16, tag=f"ri{c}")
                tt = pool.tile([H, NFC], b16, tag=f"tt{c}")
                ut = pool.tile([H, NFC], b16, tag=f"ut{c}")
                L = pool.tile([H, WC * 3 * OC], b16, tag=f"L{c}")
                R = pool.tile([H, 3 * NFC], b16, tag=f"R{c}")

                nyv = nyxt[c][:, 0:NFC]
                nxv = nyxt[c][:, NFC : 2 * NFC]
                L4 = L[:, :].rearrange("q (w f c) -> q w f c", f=3, c=OC)

                # ---- scalar chain: softplus -> sp (R feature block 0)
                nc.scalar.activation(out=ewt[:, :], in_=wtt[c][:, :], func=Act.Exp)
                nc.scalar.activation(
                    out=R[:, 0:NFC], in_=ewt[:, :], func=Act.Ln, bias=1.0
                )

                # ---- one-hot branch
                sg_wc = sgt[c][:, :].rearrange("q (w k) -> q w k", k=NCLS)
                nc.vector.tensor_reduce(
                    out=mxt[:, :], in_=sg_wc, axis=Axis.X, op=Alu.max
                )
                mx_b = mxt[:, :].unsqueeze(2).broadcast_to((H, WC, OC))
                hot = L4[:, :, 0, :]
                nc.vector.tensor_tensor(
                    out=hot, in0=sg_wc[:, :, 1:NCLS], in1=mx_b, op=Alu.is_equal
                )
                cw_r = cwbt[c][:, :].rearrange("q (w k) -> q w k", k=OC)
                nc.vector.tensor_tensor(
                    out=L4[:, :, 2, :], in0=hot, in1=cw_r, op=Alu.mult
                )
                nc.scalar.mul(out=L4[:, :, 1, :], in_=hot, mul=cht[:, :])

                # ---- direction features
                nc.vector._custom_dve(
                    RINV, out=rit[:, :], in0=nxv, in1=nyv, s0=RC0, s1=RC1
                )
                nc.vector.tensor_tensor(
                    out=tt[:, :], in0=R[:, 0:NFC], in1=rit[:, :], op=Alu.mult
                )
                nc.vector.tensor_tensor(
                    out=ut[:, :], in0=tt[:, :], in1=nxv, op=Alu.mult
                )
                # m = u*ny, R11 = u*nx fused over the adjacent [ny|nx] halves
                u_b = ut[:, :].unsqueeze(1).broadcast_to((H, 2, NFC))
                nyx_r = nyxt[c][:, :].rearrange("q (b f) -> q b f", b=2)
                mr_out = R[:, NFC : 3 * NFC].rearrange("q (b f) -> q b f", b=2)
                nc.vector.tensor_tensor(
                    out=mr_out, in0=u_b, in1=nyx_r, op=Alu.mult
                )

                # ---- segment reduce on TensorE, G columns per matmul
                Rv = R[:, :].rearrange("q (f g w) -> q w f g", f=3, g=NPTS)
                nwin = WC // G
                for wi in range(nwin):
                    nc.tensor.matmul(
                        acc[:, :],
                        L[:, wi * G * 24 : (wi + 1) * G * 24],
                        Rv[:, wi * G : (wi + 1) * G, :, :],
                        start=(c == 0 and wi == 0),
                        stop=(c == NCH - 1 and wi == nwin - 1),
                    )

            outs = pool.tile([G * 24, G * 27], f32, tag="outs")
            nc.scalar.copy(out=outs[:, :], in_=acc[:, :])
            nc.sync.dma_start(out=out_d[:, :], in_=outs[:, :])

    nc.compile()
    return nc


def _host_constants():
    import ml_dtypes

    bf16 = ml_dtypes.bfloat16
    cw = ((np.arange(W, dtype=np.float32) + 0.5) / HEIGHT).astype(bf16)
    cwb = np.ascontiguousarray(
        np.broadcast_to(cw[None, :, None], (H, W, OC))
    ).reshape(H, W * OC)
    chv = ((np.arange(H, dtype=np.float32) + 0.5) / HEIGHT).reshape(H, 1)
    return cwb, chv


def _prep_inputs(seg, direct, w):
    """Host-side sharding/staging: dtype cast + layout permutation only."""
    import ml_dtypes

    bf16 = ml_dtypes.bfloat16
    # direct [B,H,W,18] -> per-point planes, chunk-major [B,H,NCH,2,NPTS,WC]
    d = direct.reshape(B, H, NCH, WC, NPTS, 2)
    nyx = np.ascontiguousarray(
        d.transpose(0, 1, 2, 5, 4, 3)[:, :, :, ::-1, :, :]  # (ny, nx) order
    ).astype(bf16).reshape(B, H, NCH * 2 * NFC)
    wb = (
        np.ascontiguousarray(
            w.reshape(B, H, NCH, WC, NPTS).transpose(0, 1, 2, 4, 3)
        )
        .astype(bf16)
        .reshape(B, H, NCH * NFC)
    )
    segb = np.ascontiguousarray(seg.reshape(B, H, W * NCLS))
    return segb, nyx, wb


def _solve_host(a96: np.ndarray) -> np.ndarray:
    """acc [96,108] fp32 -> p [OC, NPTS, 2] fp32 (float64 pinv like ref)."""
    a = a96.astype(np.float64)
    acc = np.zeros((24, 27), dtype=np.float64)
    for j in range(G):
        acc += a[j * 24 : (j + 1) * 24, j * 27 : (j + 1) * 27]
    H0, H1, H2 = acc[0:OC], acc[OC : 2 * OC], acc[2 * OC : 3 * OC]
    SP0, M0, D0 = H0[:, 0:9], H0[:, 9:18], H0[:, 18:27]
    SP1, M1, D1 = H1[:, 0:9], H1[:, 9:18], H1[:, 18:27]
    SP2, M2, D2 = H2[:, 0:9], H2[:, 9:18], H2[:, 18:27]
    A = SP0 - D0
    Bm = M0
    D = D0
    qx = (SP1 - D1) - M2
    qy = D2 - M1
    Rm = np.empty((OC, NPTS, 2, 2), dtype=np.float64)
    Rm[..., 0, 0] = A
    Rm[..., 0, 1] = -Bm
    Rm[..., 1, 0] = -Bm
    Rm[..., 1, 1] = D
    q = np.stack([qx, qy], axis=-1)
    Rp = np.linalg.pinv(Rm.reshape(-1, 2, 2)).reshape(Rm.shape)
    p = np.einsum("cpij,cpj->cpi", Rp, q) * HEIGHT
    return p.astype(np.float32)


def kernel(seg, direct, w):
    if "nc" not in _cache:
        _cache["nc"] = _build_nc()
    nc = _cache["nc"]

    seg = np.ascontiguousarray(np.asarray(seg, dtype=np.float32))
    direct = np.ascontiguousarray(np.asarray(direct, dtype=np.float32))
    w = np.ascontiguousarray(np.asarray(w, dtype=np.float32))
    segb, nyx, wb = _prep_inputs(seg, direct, w)
    cwb, chv = _host_constants()

    in_maps = []
    for i in range(B):
        in_maps.append(
            {
                "seg": segb[i],
                "nyx": nyx[i],
                "w": wb[i],
                "cwb": cwb,
                "chv": chv,
            }
        )

    from concourse.bass_utils import run_bass_kernel_spmd

    trace = bool(int(os.environ.get("KERNEL_TRACE", "0")))
    res = run_bass_kernel_spmd(
        nc, in_maps, core_ids=list(range(N_CORES)), trace=trace
    )
    kernel._last_exec_ns = res.exec_time_ns
    kernel._last_results = res

    out = np.stack(
        [_solve_host(np.asarray(res.results[i]["acc"])) for i in range(B)], axis=0
    )
    return out
